# revision 9
# baseline (speedup 1.0000x reference)
"""Trainium2 Bass kernel for nn_Expert (MoE routing): 8-expert 2-layer MLP
with softmax gating, data-parallel over batch across 8 NeuronCores.

Strategy:
  - Shard batch B=8192 into 8 shards of 1024 rows (one per core); expert and
    gate parameters are replicated. No collectives.
  - Everything on-chip is kept "transposed" (features on partitions, batch on
    the free dim) so all weight matrices act as natural-layout stationary
    operands and biases are per-partition vectors.
  - The softmax gate weight choose[b,g] is folded into the hidden activations
    (h * choose broadcast over hid), so the second-layer matmuls of all 8
    experts accumulate directly into one PSUM tile per output tile.
  - The combined bias sum_g choose[b,g]*b2[g,c] is added via one extra K=8
    matmul (lhsT=b2, rhs=choose^T) into the same accumulation group.
  - Matmuls run in bf16 (weights/acts cast on host / on chip); accumulation is
    fp32 in PSUM; gate network is computed entirely in fp32.
"""

import numpy as np
import ml_dtypes

import concourse.bass as bass
import concourse.mybir as mybir
import concourse.tile as tile
from concourse import bacc
from concourse.masks import make_identity

B, C, G, HID, CTRL = 8192, 4096, 8, 1024, 64
NCORES = 8
BS = B // NCORES          # 1024 batch rows per core
NB = 512                  # moving free-dim block (PSUM bank = 512 fp32)
NBLK = BS // NB           # 2
P = 128
KC = C // P               # 32 contraction chunks for layer 1
NHT = HID // P            # 8 hid tiles
NCT = C // P              # 32 output c tiles

BF16 = mybir.dt.bfloat16
F32 = mybir.dt.float32


def build_program(reps=1):
    """reps>1 repeats the whole computation back-to-back inside one NEFF
    (outputs are overwritten idempotently) — used to measure device time as
    the wall-clock slope between reps variants, cancelling dispatch
    overhead."""
    nc = bacc.Bacc(None, target_bir_lowering=False)

    xT = nc.dram_tensor("xT", (C, BS), BF16, kind="ExternalInput")
    yT = nc.dram_tensor("yT", (CTRL, BS), F32, kind="ExternalInput")
    # w1r[g, ht, cp, kc, h] = w1[g, kc*128+cp, ht*128+h]
    w1r = nc.dram_tensor("w1r", (G, NHT, P, KC, P), BF16, kind="ExternalInput")
    # w2r[ct, hp, g*8+kh, c] = w2[g, kh*128+hp, ct*128+c]
    w2r = nc.dram_tensor("w2r", (NCT, P, G * NHT, P), BF16, kind="ExternalInput")
    # b1t[p, g, t] = b1[g, t*128+p]
    b1t = nc.dram_tensor("b1t", (P, G, NHT), F32, kind="ExternalInput")
    b2c = nc.dram_tensor("b2c", (G, C), BF16, kind="ExternalInput")
    wc1 = nc.dram_tensor("wc1", (CTRL, G), F32, kind="ExternalInput")
    bc1 = nc.dram_tensor("bc1", (G, 1), F32, kind="ExternalInput")
    wc2 = nc.dram_tensor("wc2", (G, G), F32, kind="ExternalInput")
    bc2 = nc.dram_tensor("bc2", (G, 1), F32, kind="ExternalInput")
    outT = nc.dram_tensor("outT", (C, BS), F32, kind="ExternalOutput")

    with tile.TileContext(nc) as tc:
        with (
            tc.tile_pool(name="xpool", bufs=1) as xpool,
            tc.tile_pool(name="hpool", bufs=1) as hpool,
            tc.tile_pool(name="w1pool", bufs=2) as w1pool,
            tc.tile_pool(name="w2pool", bufs=2) as w2pool,
            tc.tile_pool(name="spool", bufs=1) as spool,
            tc.tile_pool(name="opool", bufs=2) as opool,
            tc.tile_pool(name="gpool", bufs=1) as gpool,
            tc.tile_pool(name="ps1", bufs=2, space="PSUM") as ps1,
            tc.tile_pool(name="ps2", bufs=2, space="PSUM") as ps2,
            tc.tile_pool(name="psg", bufs=2, space="PSUM") as psg,
        ):
            # ---- constants / small params ----
            identity = gpool.tile([P, P], F32, name="identity", tag="identity")
            make_identity(nc, identity[:])
            wc1_sb = gpool.tile([CTRL, G], F32, name="wc1_sb", tag="wc1")
            nc.sync.dma_start(wc1_sb[:], wc1[:])
            bc1_sb = gpool.tile([G, 1], F32, name="bc1_sb", tag="bc1")
            nc.sync.dma_start(bc1_sb[:], bc1[:])
            wc2_sb = gpool.tile([G, G], F32, name="wc2_sb", tag="wc2")
            nc.sync.dma_start(wc2_sb[:], wc2[:])
            bc2_sb = gpool.tile([G, 1], F32, name="bc2_sb", tag="bc2")
            nc.sync.dma_start(bc2_sb[:], bc2[:])
            b1_sb = gpool.tile([P, G, NHT], F32, name="b1_sb", tag="b1")
            nc.sync.dma_start(b1_sb[:], b1t[:])
            b2_sb = gpool.tile([G, C], BF16, name="b2_sb", tag="b2")
            nc.sync.dma_start(b2_sb[:], b2c[:])
            yT_sb = gpool.tile([CTRL, BS], F32, name="yT_sb", tag="yT")
            nc.sync.dma_start(yT_sb[:], yT[:])

            # ---- resident x^T (bf16) ----
            xts = []
            for kc in range(KC):
                xt = xpool.tile([P, BS], BF16, name=f"xT_{kc}", tag=f"x{kc}")
                nc.sync.dma_start(xt[:], xT[kc * P:(kc + 1) * P, :])
                xts.append(xt)

            for rep in range(reps):
              for blk in range(NBLK):
                bsl = slice(blk * NB, (blk + 1) * NB)

                # ---- gate network (fp32): choose^T [G, NB] + bcast scales --
                g1 = psg.tile([G, NB], F32, name="g1", tag="psg")
                nc.tensor.matmul(g1[:], wc1_sb[:], yT_sb[:, bsl],
                                 start=True, stop=True)
                gh = gpool.tile([G, NB], F32, name=f"gh_{blk}", tag="gh")
                nc.scalar.activation(gh[:], g1[:],
                                     mybir.ActivationFunctionType.Relu,
                                     bias=bc1_sb[:])
                g2 = psg.tile([G, NB], F32, name="g2", tag="psg")
                nc.tensor.matmul(g2[:], wc2_sb[:], gh[:], start=True, stop=True)
                logit = gpool.tile([G, NB], F32, name=f"logit_{blk}", tag="logit")
                nc.vector.tensor_scalar_add(logit[:], g2[:], bc2_sb[:])

                # transpose logits to [b, G] chunks, softmax along free dim
                logitN = gpool.tile([P, NB // P, G], F32,
                                    name=f"logitN_{blk}", tag="logitN")
                for j in range(NB // P):
                    pt = psg.tile([P, G], F32, name=f"pt_{blk}_{j}", tag="psg")
                    nc.tensor.transpose(pt[:], logit[:, j * P:(j + 1) * P],
                                        identity[:G, :G])
                    nc.vector.tensor_copy(logitN[:, j, :], pt[:])
                mx = gpool.tile([P, NB // P], F32, name=f"mx_{blk}", tag="mx")
                nc.vector.tensor_reduce(mx[:], logitN[:],
                                        axis=mybir.AxisListType.X,
                                        op=mybir.AluOpType.max)
                negmx = gpool.tile([P, NB // P], F32, name=f"negmx_{blk}", tag="negmx")
                nc.vector.tensor_scalar_mul(negmx[:], mx[:], -1.0)
                ex = gpool.tile([P, NB // P, G], F32, name=f"ex_{blk}", tag="ex")
                for j in range(NB // P):
                    nc.scalar.activation(ex[:, j, :], logitN[:, j, :],
                                         mybir.ActivationFunctionType.Exp,
                                         bias=negmx[:, j:j + 1])
                sm = gpool.tile([P, NB // P], F32, name=f"sm_{blk}", tag="sm")
                nc.vector.tensor_reduce(sm[:], ex[:],
                                        axis=mybir.AxisListType.X,
                                        op=mybir.AluOpType.add)
                rs = gpool.tile([P, NB // P], F32, name=f"rs_{blk}", tag="rs")
                nc.vector.reciprocal(rs[:], sm[:])
                choose = gpool.tile([P, NB // P, G], F32,
                                    name=f"choose_{blk}", tag="choose")
                for j in range(NB // P):
                    nc.vector.tensor_scalar_mul(choose[:, j, :], ex[:, j, :],
                                                rs[:, j:j + 1])

                # transpose back to [G, NB] and cast to bf16
                chooseT = gpool.tile([G, NB], BF16,
                                     name=f"chooseT_{blk}", tag="chooseT")
                for j in range(NB // P):
                    ptj = psg.tile([G, P], F32, name=f"ptj_{blk}_{j}", tag="psg")
                    nc.tensor.transpose(ptj[:], choose[:, j, :], identity[:])
                    nc.vector.tensor_copy(chooseT[:, j * P:(j + 1) * P], ptj[:])

                # ---- layer 1: h^T[g] = relu(w1[g]^T x^T + b1) * choose ----
                hts = {}
                for g in range(G):
                    # broadcast this expert's gate row across 128 partitions
                    # (partition_broadcast needs a partition-0 source, so DMA
                    # the row down to partition 0 first)
                    rowg = spool.tile([1, NB], BF16, name=f"row_{blk}_{g}",
                                      tag="row", bufs=2)
                    nc.sync.dma_start(rowg[:], chooseT[g:g + 1, :])
                    sc = spool.tile([P, NB], BF16, name=f"scale_{blk}_{g}",
                                    tag="sc", bufs=2)
                    nc.gpsimd.partition_broadcast(sc[:], rowg[:])
                    for ht in range(NHT):
                        w1t = w1pool.tile([P, KC, P], BF16,
                                          name=f"w1t_{blk}_{g}_{ht}", tag="w1")
                        nc.sync.dma_start(w1t[:], w1r[g, ht])
                        ps = ps1.tile([P, NB], F32, name=f"psh_{blk}_{g}_{ht}",
                                      tag="ps1")
                        for kc in range(KC):
                            nc.tensor.matmul(ps[:], w1t[:, kc, :],
                                             xts[kc][:, bsl],
                                             start=(kc == 0), stop=(kc == KC - 1))
                        nc.scalar.activation(ps[:], ps[:],
                                             mybir.ActivationFunctionType.Relu,
                                             bias=b1_sb[:, g, ht:ht + 1])
                        htile = hpool.tile([P, NB], BF16,
                                           name=f"hT_{blk}_{g}_{ht}",
                                           tag=f"h{g}_{ht}")
                        nc.vector.tensor_mul(htile[:], ps[:], sc[:])
                        hts[(g, ht)] = htile

                # ---- layer 2: out^T[ct] = sum_g w2[g]^T h^T[g] + choose@b2 --
                for ct in range(NCT):
                    w2t = w2pool.tile([P, G * NHT, P], BF16,
                                      name=f"w2t_{blk}_{ct}", tag="w2")
                    nc.sync.dma_start(w2t[:], w2r[ct])
                    po = ps2.tile([P, NB], F32, name=f"pso_{blk}_{ct}", tag="ps2")
                    nc.tensor.matmul(po[:], b2_sb[:, ct * P:(ct + 1) * P],
                                     chooseT[:], start=True, stop=False)
                    for g in range(G):
                        for kh in range(NHT):
                            nc.tensor.matmul(
                                po[:], w2t[:, g * NHT + kh, :], hts[(g, kh)][:],
                                start=False,
                                stop=(g == G - 1 and kh == NHT - 1))
                    oc = opool.tile([P, NB], F32, name=f"oc_{blk}_{ct}", tag="oc")
                    nc.vector.tensor_copy(oc[:], po[:])
                    nc.sync.dma_start(outT[ct * P:(ct + 1) * P, bsl], oc[:])

    nc.compile()
    return nc


def prep_inputs(x, y, w1, b1, w2, b2, wc1, bc1, wc2, bc2):
    """Host-side sharding + layout. Returns per-core in_maps."""
    bf16 = ml_dtypes.bfloat16
    x = np.asarray(x, dtype=np.float32)
    y = np.asarray(y, dtype=np.float32)
    w1 = np.asarray(w1, dtype=np.float32)
    b1 = np.asarray(b1, dtype=np.float32)
    w2 = np.asarray(w2, dtype=np.float32)
    b2 = np.asarray(b2, dtype=np.float32)
    wc1 = np.asarray(wc1, dtype=np.float32)
    bc1 = np.asarray(bc1, dtype=np.float32)
    wc2 = np.asarray(wc2, dtype=np.float32)
    bc2 = np.asarray(bc2, dtype=np.float32)

    # replicated params
    w1r = w1.reshape(G, KC, P, NHT, P).transpose(0, 3, 2, 1, 4).astype(bf16)
    w1r = np.ascontiguousarray(w1r)
    w2r = w2.reshape(G, NHT, P, NCT, P).transpose(3, 2, 0, 1, 4).astype(bf16)
    w2r = np.ascontiguousarray(w2r).reshape(NCT, P, G * NHT, P)
    b1t = np.ascontiguousarray(b1.reshape(G, NHT, P).transpose(2, 0, 1))
    b2c = b2.astype(bf16)
    bc1c = bc1.reshape(G, 1)
    bc2c = bc2.reshape(G, 1)

    in_maps = []
    for c in range(NCORES):
        sl = slice(c * BS, (c + 1) * BS)
        xTc = np.ascontiguousarray(x[sl].T).astype(bf16)
        yTc = np.ascontiguousarray(y[sl].T)
        in_maps.append({
            "xT": xTc, "yT": yTc, "w1r": w1r, "w2r": w2r, "b1t": b1t,
            "b2c": b2c, "wc1": wc1, "bc1": bc1c, "wc2": wc2, "bc2": bc2c,
        })
    return in_maps


_prog = None


def kernel(**inputs):
    global _prog
    from concourse.bass_utils import run_bass_kernel_spmd
    in_maps = prep_inputs(**inputs)
    if _prog is None:
        _prog = build_program()
    res = run_bass_kernel_spmd(_prog, in_maps, core_ids=list(range(NCORES)))
    out = np.empty((B, C), dtype=np.float32)
    for c in range(NCORES):
        out[c * BS:(c + 1) * BS] = res.results[c]["outT"].T
    return out


# ---------------------------------------------------------------------------
# Benchmarking: persistent jitted runner with device-resident inputs.
# ---------------------------------------------------------------------------

def _make_runner(nc, in_maps):
    """Build (fn, device_args, fetch) for repeated execution of a compiled
    Bass program on NCORES cores via shard_map, without donation so the same
    device buffers can be reused across calls."""
    import jax
    from jax.sharding import Mesh, PartitionSpec, NamedSharding
    from jax.experimental.shard_map import shard_map
    from concourse import bass2jax
    import concourse.mybir as mybir_

    bass2jax.install_neuronx_cc_hook()
    partition_name = (nc.partition_id_tensor.name
                      if nc.partition_id_tensor else None)
    in_names, out_names, out_avals, zero_outs = [], [], [], []
    for alloc in nc.m.functions[0].allocations:
        if not isinstance(alloc, mybir_.MemoryLocationSet):
            continue
        name = alloc.memorylocations[0].name
        if alloc.kind == "ExternalInput":
            if name != partition_name:
                in_names.append(name)
        elif alloc.kind == "ExternalOutput":
            shape = tuple(alloc.tensor_shape)
            dtype = mybir_.dt.np(alloc.dtype)
            out_names.append(name)
            out_avals.append(jax.core.ShapedArray(shape, dtype))
            zero_outs.append(np.zeros(shape, dtype))
    n_params = len(in_names)
    all_in_names = list(in_names) + list(out_names)
    if partition_name is not None:
        all_in_names.append(partition_name)

    def _body(*args):
        operands = list(args)
        if partition_name is not None:
            operands.append(bass2jax.partition_id_tensor())
        outs = bass2jax._bass_exec_p.bind(
            *operands,
            out_avals=tuple(out_avals),
            in_names=tuple(all_in_names),
            out_names=tuple(out_names),
            lowering_input_output_aliases=(),
            sim_require_finite=True,
            sim_require_nnan=True,
            nc=nc,
        )
        return tuple(outs)

    devices = jax.devices()[:NCORES]
    mesh = Mesh(np.asarray(devices), ("core",))
    spec = NamedSharding(mesh, PartitionSpec("core"))
    n_outs = len(out_names)
    fn = jax.jit(shard_map(_body, mesh=mesh,
                           in_specs=(PartitionSpec("core"),) * (n_params + n_outs),
                           out_specs=(PartitionSpec("core"),) * n_outs,
                           check_rep=False),
                 keep_unused=True)
    concat_in = [
        np.concatenate([np.asarray(in_maps[c][nm]) for c in range(NCORES)], axis=0)
        for nm in in_names
    ]
    concat_zeros = [np.zeros((NCORES * z.shape[0], *z.shape[1:]), z.dtype)
                    for z in zero_outs]
    device_args = [jax.device_put(a, spec) for a in concat_in + concat_zeros]

    def fetch(out_arrs):
        return [
            {nm: np.asarray(out_arrs[i]).reshape(NCORES, *out_avals[i].shape)[c]
             for i, nm in enumerate(out_names)}
            for c in range(NCORES)
        ]

    return fn, device_args, fetch


def _build_null_program():
    nc = bacc.Bacc(None, target_bir_lowering=False)
    a = nc.dram_tensor("a", (P, P), F32, kind="ExternalInput")
    o = nc.dram_tensor("o", (P, P), F32, kind="ExternalOutput")
    with tile.TileContext(nc) as tc:
        with tc.tile_pool(name="sb", bufs=1) as sb:
            t = sb.tile([P, P], F32, name="t", tag="t")
            nc.sync.dma_start(t[:], a[:])
            nc.sync.dma_start(o[:], t[:])
    nc.compile()
    return nc


def _timed(fn, device_args, iters):
    """Median per-call wall time (each call fully synchronized)."""
    import time as _time
    import jax
    for _ in range(2):  # warmup / compile
        jax.block_until_ready(fn(*device_args))
    samples = []
    for _ in range(iters):
        t0 = _time.perf_counter()
        jax.block_until_ready(fn(*device_args))
        samples.append(_time.perf_counter() - t0)
    samples.sort()
    return samples[len(samples) // 2]


def bench(np_inputs, iters=20, reps_hi=3):
    """Estimate per-run device ns: wall-clock slope between a program with
    the body repeated reps_hi times vs once (dispatch overhead cancels)."""
    global _prog
    in_maps = prep_inputs(**np_inputs)
    if _prog is None:
        _prog = build_program()
    times = {}
    for reps, prog in ((1, _prog), (reps_hi, build_program(reps=reps_hi))):
        fn, dev_args, _ = _make_runner(prog, in_maps)
        times[reps] = _timed(fn, dev_args, iters)
        print(f"reps={reps}: median call {times[reps]*1e6:.0f} us", flush=True)
    return (times[reps_hi] - times[1]) / (reps_hi - 1) * 1e9


# revision 16
# speedup vs baseline: 106.5540x; 106.5540x over previous
"""Trainium2 Bass kernel for nn_Expert (MoE routing): 8-expert 2-layer MLP
with softmax gating, data-parallel over batch across 8 NeuronCores.

Strategy:
  - Shard batch B=8192 into 8 shards of 1024 rows (one per core); expert and
    gate parameters are replicated. No collectives.
  - Everything on-chip is kept "transposed" (features on partitions, batch on
    the free dim) so all weight matrices act as natural-layout stationary
    operands and biases are per-partition vectors.
  - The softmax gate weight choose[b,g] is folded into the hidden activations
    (h * choose broadcast over hid), so the second-layer matmuls of all 8
    experts accumulate directly into one PSUM tile per output tile.
  - The combined bias sum_g choose[b,g]*b2[g,c] is added via one extra K=8
    matmul (lhsT=b2, rhs=choose^T) into the same accumulation group.
  - Matmuls run in bf16 (weights/acts cast on host / on chip); accumulation is
    fp32 in PSUM; gate network is computed entirely in fp32.
"""

import numpy as np
import ml_dtypes

import concourse.bass as bass
import concourse.mybir as mybir
import concourse.tile as tile
from concourse import bacc
from concourse.masks import make_identity

B, C, G, HID, CTRL = 8192, 4096, 8, 1024, 64
NCORES = 8
BS = B // NCORES          # 1024 batch rows per core
NB = 512                  # moving free-dim block (PSUM bank = 512 fp32)
NBLK = BS // NB           # 2
P = 128
KC = C // P               # 32 contraction chunks for layer 1
NHT = HID // P            # 8 hid tiles
NCT = C // P              # 32 output c tiles

BF16 = mybir.dt.bfloat16
F32 = mybir.dt.float32


def build_program(reps=1, skip_wdma=False, dma_only=False):
    """reps>1 repeats the whole computation back-to-back inside one NEFF
    (outputs are overwritten idempotently) — used to measure device time as
    the wall-clock slope between reps variants, cancelling dispatch
    overhead.

    Bisection variants (timing only, wrong results):
      skip_wdma: weight DMAs replaced by one static tile per pool.
      dma_only:  only the DMA stream (no matmul/act/vector work).
    """
    nc = bacc.Bacc(None, target_bir_lowering=False)

    xT = nc.dram_tensor("xT", (C, BS), BF16, kind="ExternalInput")
    yT = nc.dram_tensor("yT", (CTRL, BS), F32, kind="ExternalInput")
    # w1r[g, ht, cp, kc, h] = w1[g, kc*128+cp, ht*128+h]
    w1r = nc.dram_tensor("w1r", (G, NHT, P, KC, P), BF16, kind="ExternalInput")
    # w2r[ct, hp, g*8+kh, c] = w2[g, kh*128+hp, ct*128+c]
    w2r = nc.dram_tensor("w2r", (NCT, P, G * NHT, P), BF16, kind="ExternalInput")
    # b1t[p, g, t] = b1[g, t*128+p]
    b1t = nc.dram_tensor("b1t", (P, G, NHT), F32, kind="ExternalInput")
    b2c = nc.dram_tensor("b2c", (G, C), BF16, kind="ExternalInput")
    wc1 = nc.dram_tensor("wc1", (CTRL, G), F32, kind="ExternalInput")
    bc1 = nc.dram_tensor("bc1", (G, 1), F32, kind="ExternalInput")
    wc2 = nc.dram_tensor("wc2", (G, G), F32, kind="ExternalInput")
    bc2 = nc.dram_tensor("bc2", (G, 1), F32, kind="ExternalInput")
    outT = nc.dram_tensor("outT", (C, BS), F32, kind="ExternalOutput")

    with tile.TileContext(nc) as tc:
        with (
            tc.tile_pool(name="xpool", bufs=1) as xpool,
            tc.tile_pool(name="hpool", bufs=1) as hpool,
            tc.tile_pool(name="w1pool", bufs=2) as w1pool,
            tc.tile_pool(name="w2pool", bufs=2) as w2pool,
            tc.tile_pool(name="spool", bufs=1) as spool,
            tc.tile_pool(name="opool", bufs=2) as opool,
            tc.tile_pool(name="gpool", bufs=1) as gpool,
            tc.tile_pool(name="ps1", bufs=3, space="PSUM") as ps1,
            tc.tile_pool(name="ps2", bufs=3, space="PSUM") as ps2,
            tc.tile_pool(name="psg", bufs=2, space="PSUM") as psg,
        ):
            # ---- constants / small params ----
            identity = gpool.tile([P, P], F32, name="identity", tag="identity")
            make_identity(nc, identity[:])
            wc1_sb = gpool.tile([CTRL, G], F32, name="wc1_sb", tag="wc1")
            nc.sync.dma_start(wc1_sb[:], wc1[:])
            bc1_sb = gpool.tile([G, 1], F32, name="bc1_sb", tag="bc1")
            nc.sync.dma_start(bc1_sb[:], bc1[:])
            wc2_sb = gpool.tile([G, G], F32, name="wc2_sb", tag="wc2")
            nc.sync.dma_start(wc2_sb[:], wc2[:])
            bc2_sb = gpool.tile([G, 1], F32, name="bc2_sb", tag="bc2")
            nc.sync.dma_start(bc2_sb[:], bc2[:])
            b1_sb = gpool.tile([P, G, NHT], F32, name="b1_sb", tag="b1")
            nc.sync.dma_start(b1_sb[:], b1t[:])
            b2_sb = gpool.tile([G, C], BF16, name="b2_sb", tag="b2")
            nc.sync.dma_start(b2_sb[:], b2c[:])
            yT_sb = gpool.tile([CTRL, BS], F32, name="yT_sb", tag="yT")
            nc.sync.dma_start(yT_sb[:], yT[:])

            # ---- resident x^T (bf16) ----
            xts = []
            for kc in range(KC):
                xt = xpool.tile([P, BS], BF16, name=f"xT_{kc}", tag=f"x{kc}")
                nc.sync.dma_start(xt[:], xT[kc * P:(kc + 1) * P, :])
                xts.append(xt)

            if skip_wdma:
                w1_static = w1pool.tile([P, KC, P], BF16, name="w1_static",
                                        tag="w1")
                nc.sync.dma_start(w1_static[:], w1r[0, 0])
                w2_static = w2pool.tile([P, G * NHT, P], BF16,
                                        name="w2_static", tag="w2")
                nc.sync.dma_start(w2_static[:], w2r[0])

            for rep in range(reps):
              for blk in range(NBLK):
                bsl = slice(blk * NB, (blk + 1) * NB)

                if dma_only:
                    for g in range(G):
                        for ht in range(NHT):
                            w1t = w1pool.tile([P, KC, P], BF16,
                                              name=f"w1t_{rep}_{blk}_{g}_{ht}",
                                              tag="w1")
                            nc.sync.dma_start(w1t[:], w1r[g, ht])
                    for ct in range(NCT):
                        w2t = w2pool.tile([P, G * NHT, P], BF16,
                                          name=f"w2t_{rep}_{blk}_{ct}", tag="w2")
                        nc.sync.dma_start(w2t[:], w2r[ct])
                    continue

                # ---- gate network (fp32): choose^T [G, NB] + bcast scales --
                g1 = psg.tile([G, NB], F32, name="g1", tag="psg")
                nc.tensor.matmul(g1[:], wc1_sb[:], yT_sb[:, bsl],
                                 start=True, stop=True)
                gh = gpool.tile([G, NB], F32, name=f"gh_{blk}", tag="gh")
                nc.scalar.activation(gh[:], g1[:],
                                     mybir.ActivationFunctionType.Relu,
                                     bias=bc1_sb[:])
                g2 = psg.tile([G, NB], F32, name="g2", tag="psg")
                nc.tensor.matmul(g2[:], wc2_sb[:], gh[:], start=True, stop=True)
                logit = gpool.tile([G, NB], F32, name=f"logit_{blk}", tag="logit")
                nc.vector.tensor_scalar_add(logit[:], g2[:], bc2_sb[:])

                # transpose logits to [b, G] chunks, softmax along free dim
                logitN = gpool.tile([P, NB // P, G], F32,
                                    name=f"logitN_{blk}", tag="logitN")
                for j in range(NB // P):
                    pt = psg.tile([P, G], F32, name=f"pt_{blk}_{j}", tag="psg")
                    nc.tensor.transpose(pt[:], logit[:, j * P:(j + 1) * P],
                                        identity[:G, :G])
                    nc.vector.tensor_copy(logitN[:, j, :], pt[:])
                mx = gpool.tile([P, NB // P], F32, name=f"mx_{blk}", tag="mx")
                nc.vector.tensor_reduce(mx[:], logitN[:],
                                        axis=mybir.AxisListType.X,
                                        op=mybir.AluOpType.max)
                negmx = gpool.tile([P, NB // P], F32, name=f"negmx_{blk}", tag="negmx")
                nc.vector.tensor_scalar_mul(negmx[:], mx[:], -1.0)
                ex = gpool.tile([P, NB // P, G], F32, name=f"ex_{blk}", tag="ex")
                for j in range(NB // P):
                    nc.scalar.activation(ex[:, j, :], logitN[:, j, :],
                                         mybir.ActivationFunctionType.Exp,
                                         bias=negmx[:, j:j + 1])
                sm = gpool.tile([P, NB // P], F32, name=f"sm_{blk}", tag="sm")
                nc.vector.tensor_reduce(sm[:], ex[:],
                                        axis=mybir.AxisListType.X,
                                        op=mybir.AluOpType.add)
                rs = gpool.tile([P, NB // P], F32, name=f"rs_{blk}", tag="rs")
                nc.vector.reciprocal(rs[:], sm[:])
                choose = gpool.tile([P, NB // P, G], F32,
                                    name=f"choose_{blk}", tag="choose")
                for j in range(NB // P):
                    nc.vector.tensor_scalar_mul(choose[:, j, :], ex[:, j, :],
                                                rs[:, j:j + 1])

                # transpose back to [G, NB] and cast to bf16
                chooseT = gpool.tile([G, NB], BF16,
                                     name=f"chooseT_{blk}", tag="chooseT")
                for j in range(NB // P):
                    ptj = psg.tile([G, P], F32, name=f"ptj_{blk}_{j}", tag="psg")
                    nc.tensor.transpose(ptj[:], choose[:, j, :], identity[:])
                    nc.vector.tensor_copy(chooseT[:, j * P:(j + 1) * P], ptj[:])

                # ---- layer 1: h^T[g] = relu(w1[g]^T x^T + b1) * choose ----
                hts = {}
                for g in range(G):
                    # broadcast this expert's gate row across 128 partitions
                    # (partition_broadcast needs a partition-0 source, so DMA
                    # the row down to partition 0 first)
                    rowg = spool.tile([1, NB], BF16, name=f"row_{blk}_{g}",
                                      tag="row", bufs=2)
                    nc.sync.dma_start(rowg[:], chooseT[g:g + 1, :])
                    sc = spool.tile([P, NB], BF16, name=f"scale_{blk}_{g}",
                                    tag="sc", bufs=2)
                    nc.gpsimd.partition_broadcast(sc[:], rowg[:])
                    for ht in range(NHT):
                        if skip_wdma:
                            w1t = w1_static
                        else:
                            w1t = w1pool.tile([P, KC, P], BF16,
                                              name=f"w1t_{blk}_{g}_{ht}",
                                              tag="w1")
                            nc.sync.dma_start(w1t[:], w1r[g, ht])
                        ps = ps1.tile([P, NB], F32, name=f"psh_{blk}_{g}_{ht}",
                                      tag="ps1")
                        for kc in range(KC):
                            nc.tensor.matmul(ps[:], w1t[:, kc, :],
                                             xts[kc][:, bsl],
                                             start=(kc == 0), stop=(kc == KC - 1))
                        nc.scalar.activation(ps[:], ps[:],
                                             mybir.ActivationFunctionType.Relu,
                                             bias=b1_sb[:, g, ht:ht + 1])
                        htile = hpool.tile([P, NB], BF16,
                                           name=f"hT_{blk}_{g}_{ht}",
                                           tag=f"h{g}_{ht}")
                        nc.vector.tensor_mul(htile[:], ps[:], sc[:])
                        hts[(g, ht)] = htile

                # ---- layer 2: out^T[ct] = sum_g w2[g]^T h^T[g] + choose@b2 --
                for ct in range(NCT):
                    if skip_wdma:
                        w2t = w2_static
                    else:
                        w2t = w2pool.tile([P, G * NHT, P], BF16,
                                          name=f"w2t_{blk}_{ct}", tag="w2")
                        nc.sync.dma_start(w2t[:], w2r[ct])
                    po = ps2.tile([P, NB], F32, name=f"pso_{blk}_{ct}", tag="ps2")
                    nc.tensor.matmul(po[:], b2_sb[:, ct * P:(ct + 1) * P],
                                     chooseT[:], start=True, stop=False)
                    for g in range(G):
                        for kh in range(NHT):
                            nc.tensor.matmul(
                                po[:], w2t[:, g * NHT + kh, :], hts[(g, kh)][:],
                                start=False,
                                stop=(g == G - 1 and kh == NHT - 1))
                    oc = opool.tile([P, NB], F32, name=f"oc_{blk}_{ct}", tag="oc")
                    nc.vector.tensor_copy(oc[:], po[:])
                    nc.sync.dma_start(outT[ct * P:(ct + 1) * P, bsl], oc[:])

    nc.compile()
    return nc


def prep_inputs(x, y, w1, b1, w2, b2, wc1, bc1, wc2, bc2):
    """Host-side sharding + layout. Returns per-core in_maps."""
    bf16 = ml_dtypes.bfloat16
    x = np.asarray(x, dtype=np.float32)
    y = np.asarray(y, dtype=np.float32)
    w1 = np.asarray(w1, dtype=np.float32)
    b1 = np.asarray(b1, dtype=np.float32)
    w2 = np.asarray(w2, dtype=np.float32)
    b2 = np.asarray(b2, dtype=np.float32)
    wc1 = np.asarray(wc1, dtype=np.float32)
    bc1 = np.asarray(bc1, dtype=np.float32)
    wc2 = np.asarray(wc2, dtype=np.float32)
    bc2 = np.asarray(bc2, dtype=np.float32)

    # replicated params
    w1r = w1.reshape(G, KC, P, NHT, P).transpose(0, 3, 2, 1, 4).astype(bf16)
    w1r = np.ascontiguousarray(w1r)
    w2r = w2.reshape(G, NHT, P, NCT, P).transpose(3, 2, 0, 1, 4).astype(bf16)
    w2r = np.ascontiguousarray(w2r).reshape(NCT, P, G * NHT, P)
    b1t = np.ascontiguousarray(b1.reshape(G, NHT, P).transpose(2, 0, 1))
    b2c = b2.astype(bf16)
    bc1c = bc1.reshape(G, 1)
    bc2c = bc2.reshape(G, 1)

    in_maps = []
    for c in range(NCORES):
        sl = slice(c * BS, (c + 1) * BS)
        xTc = np.ascontiguousarray(x[sl].T).astype(bf16)
        yTc = np.ascontiguousarray(y[sl].T)
        in_maps.append({
            "xT": xTc, "yT": yTc, "w1r": w1r, "w2r": w2r, "b1t": b1t,
            "b2c": b2c, "wc1": wc1, "bc1": bc1c, "wc2": wc2, "bc2": bc2c,
        })
    return in_maps


_prog = None


def kernel(**inputs):
    global _prog
    import time as _time
    from concourse.bass_utils import run_bass_kernel_spmd
    in_maps = prep_inputs(**inputs)
    if _prog is None:
        _prog = build_program()
    res = None
    for attempt in range(3):
        try:
            res = run_bass_kernel_spmd(_prog, in_maps,
                                       core_ids=list(range(NCORES)))
            break
        except Exception:
            if attempt == 2:
                raise
            _time.sleep(10)  # transient axon mesh desync — retry
    out = np.empty((B, C), dtype=np.float32)
    for c in range(NCORES):
        out[c * BS:(c + 1) * BS] = res.results[c]["outT"].T
    return out


# ---------------------------------------------------------------------------
# Benchmarking: persistent jitted runner with device-resident inputs.
# ---------------------------------------------------------------------------

def _make_runner(nc, in_maps):
    """Build (fn, device_args, fetch) for repeated execution of a compiled
    Bass program on NCORES cores via shard_map, without donation so the same
    device buffers can be reused across calls."""
    import jax
    from jax.sharding import Mesh, PartitionSpec, NamedSharding
    from jax.experimental.shard_map import shard_map
    from concourse import bass2jax
    import concourse.mybir as mybir_

    bass2jax.install_neuronx_cc_hook()
    partition_name = (nc.partition_id_tensor.name
                      if nc.partition_id_tensor else None)
    in_names, out_names, out_avals, zero_outs = [], [], [], []
    for alloc in nc.m.functions[0].allocations:
        if not isinstance(alloc, mybir_.MemoryLocationSet):
            continue
        name = alloc.memorylocations[0].name
        if alloc.kind == "ExternalInput":
            if name != partition_name:
                in_names.append(name)
        elif alloc.kind == "ExternalOutput":
            shape = tuple(alloc.tensor_shape)
            dtype = mybir_.dt.np(alloc.dtype)
            out_names.append(name)
            out_avals.append(jax.core.ShapedArray(shape, dtype))
            zero_outs.append(np.zeros(shape, dtype))
    n_params = len(in_names)
    all_in_names = list(in_names) + list(out_names)
    if partition_name is not None:
        all_in_names.append(partition_name)

    def _body(*args):
        operands = list(args)
        if partition_name is not None:
            operands.append(bass2jax.partition_id_tensor())
        outs = bass2jax._bass_exec_p.bind(
            *operands,
            out_avals=tuple(out_avals),
            in_names=tuple(all_in_names),
            out_names=tuple(out_names),
            lowering_input_output_aliases=(),
            sim_require_finite=True,
            sim_require_nnan=True,
            nc=nc,
        )
        return tuple(outs)

    devices = jax.devices()[:NCORES]
    mesh = Mesh(np.asarray(devices), ("core",))
    spec = NamedSharding(mesh, PartitionSpec("core"))
    n_outs = len(out_names)
    fn = jax.jit(shard_map(_body, mesh=mesh,
                           in_specs=(PartitionSpec("core"),) * (n_params + n_outs),
                           out_specs=(PartitionSpec("core"),) * n_outs,
                           check_rep=False),
                 keep_unused=True)
    concat_in = [
        np.concatenate([np.asarray(in_maps[c][nm]) for c in range(NCORES)], axis=0)
        for nm in in_names
    ]
    concat_zeros = [np.zeros((NCORES * z.shape[0], *z.shape[1:]), z.dtype)
                    for z in zero_outs]
    device_args = [jax.device_put(a, spec) for a in concat_in + concat_zeros]

    def fetch(out_arrs):
        return [
            {nm: np.asarray(out_arrs[i]).reshape(NCORES, *out_avals[i].shape)[c]
             for i, nm in enumerate(out_names)}
            for c in range(NCORES)
        ]

    return fn, device_args, fetch


def _build_null_program():
    nc = bacc.Bacc(None, target_bir_lowering=False)
    a = nc.dram_tensor("a", (P, P), F32, kind="ExternalInput")
    o = nc.dram_tensor("o", (P, P), F32, kind="ExternalOutput")
    with tile.TileContext(nc) as tc:
        with tc.tile_pool(name="sb", bufs=1) as sb:
            t = sb.tile([P, P], F32, name="t", tag="t")
            nc.sync.dma_start(t[:], a[:])
            nc.sync.dma_start(o[:], t[:])
    nc.compile()
    return nc


def _timed(fn, device_args, iters):
    """Median per-call wall time (each call fully synchronized)."""
    import time as _time
    import jax
    for _ in range(2):  # warmup / compile
        jax.block_until_ready(fn(*device_args))
    samples = []
    for _ in range(iters):
        t0 = _time.perf_counter()
        jax.block_until_ready(fn(*device_args))
        samples.append(_time.perf_counter() - t0)
    samples.sort()
    return samples[len(samples) // 2]


def bench(np_inputs, iters=20, reps_hi=3):
    """Estimate per-run device ns as the wall-clock difference between a
    program with the body repeated reps_hi times and one with it once.
    Calls to the two programs are interleaved and differenced pairwise so
    both dispatch overhead and slow drift in tunnel latency cancel."""
    import time as _time
    import jax
    global _prog
    in_maps = prep_inputs(**np_inputs)
    if _prog is None:
        _prog = build_program()
    fn1, dev1, _ = _make_runner(_prog, in_maps)
    fn3, dev3, _ = _make_runner(build_program(reps=reps_hi), in_maps)
    for _ in range(2):
        jax.block_until_ready(fn1(*dev1))
        jax.block_until_ready(fn3(*dev3))
    diffs = []
    for _ in range(iters):
        t0 = _time.perf_counter()
        jax.block_until_ready(fn1(*dev1))
        t1 = _time.perf_counter()
        jax.block_until_ready(fn3(*dev3))
        t2 = _time.perf_counter()
        diffs.append((t2 - t1) - (t1 - t0))
    diffs.sort()
    med = diffs[len(diffs) // 2]
    print(f"pairwise diff median {med*1e6:.0f} us over {iters} pairs "
          f"(p25 {diffs[len(diffs)//4]*1e6:.0f}, "
          f"p75 {diffs[3*len(diffs)//4]*1e6:.0f})", flush=True)
    return med / (reps_hi - 1) * 1e9
